# revision 27
# baseline (speedup 1.0000x reference)
"""Trainium2 Bass kernel for nn_EnsembleDynamicModel.

Ensemble MLP: E=7 members, x=[state(32)|action(8)] -> 256 -> 256 -> 256 -> 128
-> {mu(32), log_sigma(32)} with swish hidden activations, soft-clamped
log_sigma -> sigma=exp(.), and mu += state residual.

Strategy: data-parallel over the batch axis; 8 cores x 4096 rows, weights
replicated. Activations are feature-major ([feature, batch]) so every GEMM's
contraction dim sits on SBUF partitions.

The scalar (ACT) engine is the intrinsic bottleneck of a pure-ACT swish
design: 25.7M hidden elements/core at 1 elem/cycle/lane @1.2GHz = 167us,
above the PE's ~157us of matmul work, so engines must share the swish load.
A custom 8-op DVE instruction computes an approximate swish in one pass:

    out = relu(u) - |u| * relu(C1 - |u|)^2      (u = sqrt(K) * x)

which equals sqrt(K)*(relu(x) - K*|x|*relu(c-|x|)^2), a 1-knot piecewise-
quadratic swish (max abs err 0.039, and preacts here stay in |x|<3.6 where
the fit is tight; end-to-end scale_rel ~2.5e-3 vs the 2e-2 gate). The
sqrt(K) scales fold into the host-side weights: this layer's W,b scaled up,
next layer's W scaled down, so the hardware op needs zero spare ALU stages
for scaling.

Work split per ensemble (per core):
  DVE : L0's 4 drain tiles (bias rides a ones-row: K=40->41 costs nothing,
        matmul time is N-driven) + L1's mt0 tiles (bias injected via a K=1
        ones-matmul into PSUM) + the head affine drains.  ~19us
  ACT : L1 mt1 + L2 + L3 exact Silu drains + the sigma tanh.  ~20us
  PE  : the GEMM chain + K=1 bias matmuls.  ~24us  <- critical engine
  DMA : weights/input/output + the sigma pk pack copies (SBUF->SBUF DMA
        replaces cross-partition DVE copies).

Drains alternate DVE/ACT down the ensemble so no engine sees a >2-tile
burst and the PE (2-deep PSUM ring) never waits on a busy drain engine;
L0(e+1) is interleaved into L1/L2(e) to fill the PE's drain-latency gaps.
"""

import os
import sys
import numpy as np
from contextlib import ExitStack

for _p in ("/opt/trn_rl_repo", "/root/.axon_site/_ro/trn_rl_repo"):
    if os.path.isdir(_p) and _p not in sys.path:
        sys.path.append(_p)

import ml_dtypes  # noqa: E402
import concourse.bass as bass  # noqa: E402
import concourse.tile as tile  # noqa: E402
import concourse.mybir as mybir  # noqa: E402
from concourse import bacc  # noqa: E402
from concourse.bass_utils import run_bass_kernel_spmd  # noqa: E402

F32 = mybir.dt.float32
AF = mybir.ActivationFunctionType
STORE = mybir.dt.bfloat16
NP_STORE = ml_dtypes.bfloat16

E = 7
B = 32768
S = 32
A = 8
DIN = S + A + 1        # 40 real rows + ones row (bias via GEMM)
NCORES = 8
BL = B // NCORES       # 4096 batch rows per core
CH = 512               # batch chunk per psum tile (1 PSUM bank fp32)
NSUB = 512             # one matmul's free dim (1 PSUM bank fp32)
NCHUNK = BL // CH      # 8
NJ = CH // NSUB        # 1
NPS = 8                # psum ring depth: 8 x 1 bank = all 8 banks; a ring
                       # this deep hides several microseconds of drain-queue
                       # latency behind 7 tiles of matmul work
NCONST = 8             # const columns per ensemble member

# 1-knot piecewise-quadratic swish fit: s(x) ~ relu(x) - K|x|relu(c-|x|)^2
SW_K = 0.015506
SW_C = 4.9265
SW_S = float(np.sqrt(SW_K))    # layer pre-scale (folded into weights)
SW_C1 = SW_S * SW_C            # knot position in u = SW_S*x units


def _register_swish_op():
    """One 8-op custom DVE instruction: out = relu(u) - |u|*relu(C1-|u|)^2."""
    from concourse import dve_ops as dvo
    from concourse.dve_spec import (
        Spec, Src0, C1, Zero, relu, sq, maxx, lower, _has_src1,
    )
    from concourse.dve_uop import DveOpSpec

    name = "SWISH_SC_ANT"
    for op in dvo.OPS:
        if op.name == name:
            return op
    n = Zero - Src0
    a = maxx(Src0, n)
    body = relu(Src0) - a * sq(relu(C1 - a))
    spec = Spec(
        body=body,
        reference=lambda in0, in1, s0, s1, imm2: (
            np.maximum(in0, 0.0).astype(np.float32)
            - np.abs(in0) * np.maximum(s1 - np.abs(in0), 0.0) ** 2
        ).astype(np.float32),
    )
    row = dvo._CUSTOM_DVE_ROW_BASE + len(dvo.OPS)
    tmp = DveOpSpec(name=name, opcode=row, uops=lower(spec, ver="v3"),
                    rd1_en=_has_src1(spec))
    op = dvo.DveOp(name, spec, subdim=False, uops_sha={"v3": tmp.sha("v3")})
    dvo.OPS.append(op)
    dvo._SUB_OPCODE_FOR_NAME[name] = row
    dvo.CUSTOM_DVE_SPECS[name] = spec
    return op


SWISH_OP = _register_swish_op()


def _build_kernel(ctx, tc, io, act=AF.Silu):
    nc = tc.nc
    cpool = ctx.enter_context(tc.tile_pool(name="cpool", bufs=1))
    hpool = ctx.enter_context(tc.tile_pool(name="hpool", bufs=1))
    # bufs=3: wh of ensemble e-1 stays live through e's ladder (deferred
    # heads) while e+1's weights stream in
    wpool = ctx.enter_context(tc.tile_pool(name="wpool", bufs=3))
    pspool = ctx.enter_context(tc.tile_pool(name="pspool", bufs=NPS, space="PSUM"))
    sgpool = ctx.enter_context(tc.tile_pool(name="sgpool", bufs=3))

    def load_weights(e, first=False):
        # w0 packed for row-tiling: rows 0-40 = W0 cols 0:128 (array rows
        # 0-40), rows 64-104 = W0 cols 128:256 (array rows 64-104); the two
        # matmuls run concurrently in disjoint row groups.
        w0 = wpool.tile([105, 128], STORE, tag="w0", name="w0")
        nc.sync.dma_start(w0[:], io["w0"][e])
        if first:
            nc.sync.dma_start(cns[:], io["cns"])
            nc.sync.dma_start(sgc[:], io["sgc"])
            for j in range(BL // NSUB):
                js = slice(j * NSUB, (j + 1) * NSUB)
                nc.sync.dma_start(xt[0:DIN, js], io["xt"][:, js])
                nc.sync.dma_start(xt[64:64 + DIN, js], io["xt"][:, js])
        b1t = wpool.tile([1, 128], STORE, tag="b1t", name="b1t")
        nc.sync.dma_start(b1t[:], io["b1t"][e])
        w1, w2, w3 = [], [], []
        for k in range(2):
            t = wpool.tile([128, 256], STORE, tag=f"w1_{k}", name=f"w1_{k}")
            nc.sync.dma_start(t[:], io["w1"][e, k * 128:(k + 1) * 128, :])
            w1.append(t)
            t = wpool.tile([128, 256], STORE, tag=f"w2_{k}", name=f"w2_{k}")
            nc.sync.dma_start(t[:], io["w2"][e, k * 128:(k + 1) * 128, :])
            w2.append(t)
            t = wpool.tile([128, 128], STORE, tag=f"w3_{k}", name=f"w3_{k}")
            nc.sync.dma_start(t[:], io["w3"][e, k * 128:(k + 1) * 128, :])
            w3.append(t)
        wh = wpool.tile([128, 64], STORE, tag="wh", name="wh")
        nc.sync.dma_start(wh[:], io["wh"][e])
        if first:
            # 1 MB residual tensor last: not read until the first head (~25us)
            nc.sync.dma_start(resid[:], io["resid"])
        return w0, w1, w2, w3, wh, b1t

    scratch = cpool.tile([1, 8], F32, tag="scratch")
    nc.gpsimd.memset(scratch[:], 0.0)
    nc.scalar.activation(scratch[0:1, 0:8], scratch[0:1, 0:8], act, bias=0.0)

    # xt duplicated at partitions 64-104 so the row-tiled L0 pair can
    # stream both rhs copies concurrently
    xt = cpool.tile([64 + DIN, BL], STORE, tag="xt")
    cns = cpool.tile([128, E * NCONST], F32, tag="cns")
    sgc = cpool.tile([128, 2], F32, tag="sgc")
    # resid in paired-head layout: rows 0-63 = even 512-col blocks,
    # rows 64-127 = odd blocks (state on rows 0-31 / 64-95)
    resid = cpool.tile([128, BL // 2], F32, tag="resid")
    ones = cpool.tile([1, NSUB], STORE, tag="ones")
    nc.gpsimd.memset(ones[:], 1.0)

    # sigma pre-activations packed 4 ensembles per tile: row 32*(e%4)+i
    pk = [sgpool.tile([128, BL], F32, tag=f"pk{g}", name=f"pk{g}", bufs=1)
          for g in range(2)]

    # --- activation buffers ---
    # h1: double-buffered by ensemble parity (L0(e+1) interleaves with e).
    h1 = [[hpool.tile([128, BL], STORE, tag=f"h1_{p}_{i}", name=f"h1_{p}_{i}")
           for i in range(2)] for p in range(2)]
    h2 = [hpool.tile([128, BL], STORE, tag=f"h2_{i}", name=f"h2_{i}")
          for i in range(2)]
    # h3 double-buffered: heads run one ensemble deferred, so their input
    # drains are long complete and the head matmuls never wait on ACT
    h3 = [hpool.tile([128, BL], STORE, tag=f"h3_{p}", name=f"h3_{p}")
          for p in range(2)]

    def mm_tile(h_in, lhsTs, c, bias_lhsT=None, nsub=NJ):
        """Emit the matmuls for one [128, CH] psum tile; return the tile.

        kt-outer order: consecutive matmuls share their stationary operand,
        so every other LDWEIGHTS hits the already-loaded weight set."""
        ps = pspool.tile([128, CH], F32, tag="ps", name="ps")
        nkt = len(lhsTs)
        if bias_lhsT is not None:
            for j in range(nsub):
                nc.tensor.matmul(ps[:, j * NSUB:(j + 1) * NSUB],
                                 bias_lhsT[:, :], ones[:, :],
                                 start=True, stop=False, skip_group_check=True)
        for kt in range(nkt):
            for j in range(nsub):
                ncol = slice(c * CH + j * NSUB, c * CH + (j + 1) * NSUB)
                nc.tensor.matmul(
                    ps[:, j * NSUB:(j + 1) * NSUB], lhsTs[kt],
                    h_in[kt][:, ncol],
                    start=(bias_lhsT is None and kt == 0),
                    stop=(kt == nkt - 1),
                    skip_group_check=True,
                )
        return ps

    def l0_pair(w0, c, out):
        """Both 128-wide halves of L0 run concurrently as row-tiles (the
        K=41 contraction only occupies rows 0-40 / 64-104 of the array)."""
        psA = pspool.tile([128, CH], F32, tag="ps", name="psA")
        psB = pspool.tile([128, CH], F32, tag="ps", name="psB")
        for j in range(NJ):
            js = slice(j * NSUB, (j + 1) * NSUB)
            ncol = slice(c * CH + j * NSUB, c * CH + (j + 1) * NSUB)
            nc.tensor.matmul(psA[:, js], w0[0:DIN, :], xt[0:DIN, ncol],
                             start=True, stop=True, skip_group_check=True)
            nc.tensor.matmul(psB[:, js], w0[64:64 + DIN, :],
                             xt[64:64 + DIN, ncol],
                             start=True, stop=True, skip_group_check=True)
        drain_dve(psA, out[0][:, c * CH:(c + 1) * CH])
        drain_dve(psB, out[1][:, c * CH:(c + 1) * CH])

    def drain_act(ps, out_ap, bcol):
        nc.scalar.activation(out_ap, ps[:, :], act, bias=cns[:, bcol:bcol + 1])

    def drain_dve(ps, out_ap):
        nc.vector._custom_dve(SWISH_OP, out=out_ap, in0=ps[:, :], s1=SW_C1)

    def head_pair(e, wh, p):
        """Heads for 512-col subchunks 2p and 2p+1 run concurrently as
        col-tiles: psum rows 0-63 = subchunk 2p, rows 64-127 = 2p+1.
        M=64 only fills half the array's columns, so pairing halves the
        head's PE stream time."""
        h3e = h3[e % 2]
        s0 = slice(2 * p * NSUB, (2 * p + 1) * NSUB)
        s1 = slice((2 * p + 1) * NSUB, (2 * p + 2) * NSUB)
        ps = pspool.tile([128, NSUB], F32, tag="ps", name="psh")
        nc.tensor.matmul(ps[0:64, :], wh[:, :], h3e[:, s0],
                         start=True, stop=True, skip_group_check=True)
        nc.tensor.matmul(ps[64:128, :], wh[:, :], h3e[:, s1],
                         start=True, stop=True, skip_group_check=True,
                         tile_position=(0, 64))
        # one fused DVE op drains both head psums (resid is pre-packed in
        # the same row layout):
        #   rows 0-31/64-95:   mu = psum + bmu + state
        #   rows 32-63/96-127: y' = psum + (bsig - max) + 0
        bcol = e * NCONST + 7
        hd = sgpool.tile([128, NSUB], F32, tag="hd", name="hd")
        nc.vector.affine_then_add(
            hd[:, :], ps[:, :], resid[:, p * NSUB:(p + 1) * NSUB], 1.0,
            cns[:, bcol:bcol + 1],
        )
        mu_rows = e * 32
        nc.sync.dma_start(io["mu"][mu_rows:mu_rows + 32, s0], hd[0:32, :])
        nc.sync.dma_start(io["mu"][mu_rows:mu_rows + 32, s1], hd[64:96, :])
        # sigma = 0.5*tanh(y'/2)*exp(max)*2... via the pk pack: members
        # stack 4-wide so each tanh covers up to 128 ACT lanes. SBUF->SBUF
        # DMA does the cross-partition pack (frees the DVE).
        g, r = divmod(e, 4)
        nc.sync.dma_start(pk[g][r * 32:(r + 1) * 32, s0], hd[32:64, :])
        nc.sync.dma_start(pk[g][r * 32:(r + 1) * 32, s1], hd[96:128, :])
        if e in (3, E - 1):
            # flush the pk group: e=3 -> members 0-3, e=6 -> members 4-6
            rows = 128 if e == 3 else 96
            for ss in (s0, s1):
                sg2 = sgpool.tile([128, NSUB], F32, tag="sg2", name="sg2",
                                  bufs=2)
                nc.scalar.activation(sg2[0:rows, :], pk[g][0:rows, ss],
                                     AF.Tanh, scale=0.5)
                sg3 = sgpool.tile([128, NSUB], F32, tag="sg3", name="sg3",
                                  bufs=2)
                nc.vector.tensor_scalar(
                    sg3[0:rows, :], sg2[0:rows, :],
                    sgc[0:rows, 0:1], sgc[0:rows, 1:2],
                    mybir.AluOpType.mult, mybir.AluOpType.add,
                )
                nc.sync.dma_start(io["sig"][g * 128:g * 128 + rows, ss],
                                  sg3[0:rows, :])

    # --- the pipeline ---
    w_cur = None
    for e in range(E):
        if e == 0:
            w_cur = load_weights(0, first=True)
            for c in range(NCHUNK):
                l0_pair(w_cur[0], c, h1[0])
        w0, w1, w2, w3, wh, b1t = w_cur
        h1c = h1[e % 2]
        h1n = h1[(e + 1) % 2]
        cn = e * NCONST
        w_nxt = load_weights(e + 1) if e < E - 1 else None

        def l1_mt0(c):
            ps = mm_tile(h1c, [w1[0][:, 0:128], w1[1][:, 0:128]], c,
                         bias_lhsT=b1t)
            drain_dve(ps, h2[0][:, c * CH:(c + 1) * CH])

        def l1_mt1(c):
            ps = mm_tile(h1c, [w1[0][:, 128:256], w1[1][:, 128:256]], c)
            drain_act(ps, h2[1][:, c * CH:(c + 1) * CH], cn + 3)

        def l0n(c):
            if e == E - 1:
                return
            l0_pair(w_nxt[0], c, h1n)

        def l2(c, mt):
            ps = mm_tile(h2, [w2[0][:, mt * 128:(mt + 1) * 128],
                              w2[1][:, mt * 128:(mt + 1) * 128]], c)
            # L2 writes back into h1c (free after the L1(e) matmuls read it)
            drain_act(ps, h1c[mt][:, c * CH:(c + 1) * CH], cn + 4 + mt)

        def l3(c):
            ps = mm_tile(h1c, [w3[0], w3[1]], c)
            drain_act(ps, h3[e % 2][:, c * CH:(c + 1) * CH], cn + 6)

        # Chunk-pipelined ladder: layer L of chunk c runs alongside layer
        # L+1 of chunk c-1, so DVE drains (l1_mt0/l0n/head) and ACT drains
        # (l1_mt1/l2/l3) alternate evenly and no drain engine ever sees a
        # long same-engine burst. Heads of e-1 ride this ensemble's ladder:
        # their h3 drained an ensemble ago, so the head matmuls never wait.
        for c in range(NCHUNK):
            l1_mt0(c)                               # D
            l1_mt1(c)                               # A
            if e > 0 and c % 2 == 0:
                head_pair(e - 1, wh_prev, c // 2)   # D
            l0n(c)                                  # DD
            if c >= 1:
                l2(c - 1, 0)                        # A
                l2(c - 1, 1)                        # A
            if c >= 2:
                l3(c - 2)                           # A
        l2(NCHUNK - 1, 0)                           # A
        l2(NCHUNK - 1, 1)                           # A
        l3(NCHUNK - 2)                              # A
        l3(NCHUNK - 1)                              # A
        wh_prev = wh
        w_cur = w_nxt

    # epilogue: the last ensemble's heads (+ the pk group-1 sigma flush)
    for p in range(BL // (2 * NSUB)):
        head_pair(E - 1, wh_prev, p)


def build_program(act=AF.Silu):
    nc = bacc.Bacc(
        "TRN2", target_bir_lowering=False, debug=False, num_devices=NCORES
    )
    io = {
        "xt": nc.dram_tensor("xt", [DIN, BL], STORE,
                             kind="ExternalInput").ap(),
        "resid": nc.dram_tensor("resid", [128, BL // 2], F32,
                                kind="ExternalInput").ap(),
        "w0": nc.dram_tensor("w0", [E, 105, 128], STORE,
                             kind="ExternalInput").ap(),
        "w1": nc.dram_tensor("w1", [E, 256, 256], STORE,
                             kind="ExternalInput").ap(),
        "w2": nc.dram_tensor("w2", [E, 256, 256], STORE,
                             kind="ExternalInput").ap(),
        "w3": nc.dram_tensor("w3", [E, 256, 128], STORE,
                             kind="ExternalInput").ap(),
        "wh": nc.dram_tensor("wh", [E, 128, 64], STORE,
                             kind="ExternalInput").ap(),
        "b1t": nc.dram_tensor("b1t", [E, 1, 128], STORE,
                              kind="ExternalInput").ap(),
        "cns": nc.dram_tensor("cns", [128, E * NCONST], F32,
                              kind="ExternalInput").ap(),
        "sgc": nc.dram_tensor("sgc", [128, 2], F32, kind="ExternalInput").ap(),
        "mu": nc.dram_tensor("mu", [E * 32, BL], F32,
                             kind="ExternalOutput").ap(),
        "sig": nc.dram_tensor("sig", [E * 32, BL], F32,
                              kind="ExternalOutput").ap(),
    }
    with tile.TileContext(nc) as tc, ExitStack() as ctx:
        _build_kernel(ctx, tc, io, act=act)
    nc.compile()
    return nc


def host_prep(state, action, W0, b0, W1, b1, W2, b2, W3, b3,
              Wmu, bmu, Wsig, bsig, max_logstd, min_logstd):
    """Full inputs -> (shared input map, per-core shard maps)."""
    f = lambda a: np.ascontiguousarray(np.asarray(a), dtype=np.float32)
    g = lambda a: np.ascontiguousarray(np.asarray(a, dtype=np.float32)
                                       .astype(NP_STORE))
    state, action = f(state), f(action)
    x_full = np.concatenate([state, action], axis=1)          # [B, 40]
    xt_full = np.concatenate(
        [x_full.T, np.ones((1, B), np.float32)], axis=0
    )  # [41, B] with the ones row for GEMM-side biases
    W0, W1, W2, W3 = f(W0), f(W1), f(W2), f(W3)
    b0, b1, b2, b3 = f(b0), f(b1), f(b2), f(b3)
    bmu, bsig = f(bmu), f(bsig)
    mx, mn = f(max_logstd), f(min_logstd)

    s = SW_S
    # L0 folds its bias into the ones-row and pre-scales by s; the DVE swish
    # then emits s*h1, compensated in W1. L1's mt0 half does the same (its
    # bias rides a K=1 matmul), compensated in W2's first k-tile.
    W0a = np.concatenate([W0, b0[:, None, :]], axis=1) * s     # [E,41,256]
    # packed for row-tiled pairing: rows 0-40 = cols 0:128, rows 64-104 =
    # cols 128:256
    W0p = np.zeros((E, 105, 128), np.float32)
    W0p[:, 0:41, :] = W0a[:, :, 0:128]
    W0p[:, 64:105, :] = W0a[:, :, 128:256]
    W1p = np.concatenate([W1[:, :, 0:128],                     # *(s/s) = 1
                          W1[:, :, 128:256] / s], axis=2)
    b1t = (b1[:, None, 0:128] * s)                             # [E,1,128]
    W2p = np.concatenate([W2[:, 0:128, :] / s, W2[:, 128:256, :]], axis=1)
    wh = np.concatenate([f(Wmu), f(Wsig)], axis=2)

    cns = np.zeros((128, E * NCONST), np.float32)
    for e in range(E):
        c = e * NCONST
        cns[:, c + 3] = b1[e, 128:]
        cns[:, c + 4] = b2[e, :128]
        cns[:, c + 5] = b2[e, 128:]
        cns[:, c + 6] = b3[e, :]
        cns[0:32, c + 7] = bmu[e]
        cns[32:64, c + 7] = bsig[e] - mx   # sigma-head drain bias
        cns[64:96, c + 7] = bmu[e]         # paired-head upper half
        cns[96:128, c + 7] = bsig[e] - mx

    sgc = np.zeros((128, 2), np.float32)
    sgc[:, 0] = np.tile(np.exp(mx) / 2, 4)
    sgc[:, 1] = np.tile(np.exp(mn) + np.exp(mx) / 2, 4)

    shared = {
        "w0": g(W0p), "w1": g(W1p), "w2": g(W2p), "w3": g(W3),
        "wh": g(wh), "b1t": g(b1t), "cns": cns, "sgc": sgc,
    }
    # resid in paired-head layout: [128, B/2] with even 512-col blocks on
    # rows 0-31 and odd blocks on rows 64-95 (state only; sigma rows stay 0)
    st = xt_full[0:32].reshape(32, B // 512, 512)
    resid_full = np.zeros((128, B // 2), np.float32)
    resid_full[0:32] = st[:, 0::2, :].reshape(32, B // 2)
    resid_full[64:96] = st[:, 1::2, :].reshape(32, B // 2)
    xt_store = xt_full.astype(NP_STORE)
    hb = BL // 2
    shards = [
        {
            "xt": np.ascontiguousarray(xt_store[:, c * BL:(c + 1) * BL]),
            "resid": np.ascontiguousarray(resid_full[:, c * hb:(c + 1) * hb]),
        }
        for c in range(NCORES)
    ]
    return shared, shards


def host_post(results):
    """Per-core {mu,sig} [E*32, BL] -> (mu [E,B,32], sigma [E,B,32])."""
    mu = np.empty((E, B, 32), np.float32)
    sigma = np.empty((E, B, 32), np.float32)
    for c in range(NCORES):
        bs = slice(c * BL, (c + 1) * BL)
        mu[:, bs, :] = results[c]["mu"].reshape(E, 32, BL).transpose(0, 2, 1)
        sigma[:, bs, :] = results[c]["sig"].reshape(E, 32, BL).transpose(0, 2, 1)
    return mu, sigma


_PROGRAM = None


def _get_program():
    global _PROGRAM
    if _PROGRAM is None:
        _PROGRAM = build_program()
    return _PROGRAM


def kernel(**inputs):
    nc = _get_program()
    shared, shards = host_prep(**inputs)
    in_maps = [{**shared, **shards[c]} for c in range(NCORES)]
    res = run_bass_kernel_spmd(nc, in_maps, list(range(NCORES)))
    return host_post(res.results)


# revision 32
# speedup vs baseline: 1.2068x; 1.2068x over previous
"""Trainium2 Bass kernel for nn_EnsembleDynamicModel.

Ensemble MLP: E=7 members, x=[state(32)|action(8)] -> 256 -> 256 -> 256 -> 128
-> {mu(32), log_sigma(32)} with swish hidden activations, soft-clamped
log_sigma -> sigma=exp(.), and mu += state residual.

Strategy: data-parallel over the batch axis; 8 cores x 4096 rows, weights
replicated. Activations are feature-major ([feature, batch]) so every GEMM's
contraction dim sits on SBUF partitions.

The scalar (ACT) engine is the intrinsic bottleneck of a pure-ACT swish
design: 25.7M hidden elements/core at 1 elem/cycle/lane @1.2GHz = 167us,
above the PE's ~157us of matmul work, so engines must share the swish load.
A custom 8-op DVE instruction computes an approximate swish in one pass:

    out = relu(u) - |u| * relu(C1 - |u|)^2      (u = sqrt(K) * x)

which equals sqrt(K)*(relu(x) - K*|x|*relu(c-|x|)^2), a 1-knot piecewise-
quadratic swish (max abs err 0.039, and preacts here stay in |x|<3.6 where
the fit is tight; end-to-end scale_rel ~2.5e-3 vs the 2e-2 gate). The
sqrt(K) scales fold into the host-side weights: this layer's W,b scaled up,
next layer's W scaled down, so the hardware op needs zero spare ALU stages
for scaling.

Work split per ensemble (per core):
  DVE : L0's 4 drain tiles (bias rides a ones-row: K=40->41 costs nothing,
        matmul time is N-driven) + L1's mt0 tiles (bias injected via a K=1
        ones-matmul into PSUM) + the head affine drains.  ~19us
  ACT : L1 mt1 + L2 + L3 exact Silu drains + the sigma tanh.  ~20us
  PE  : the GEMM chain + K=1 bias matmuls.  ~24us  <- critical engine
  DMA : weights/input/output + the sigma pk pack copies (SBUF->SBUF DMA
        replaces cross-partition DVE copies).

Drains alternate DVE/ACT down the ensemble so no engine sees a >2-tile
burst and the PE (2-deep PSUM ring) never waits on a busy drain engine;
L0(e+1) is interleaved into L1/L2(e) to fill the PE's drain-latency gaps.
"""

import os
import sys
import numpy as np
from contextlib import ExitStack

for _p in ("/opt/trn_rl_repo", "/root/.axon_site/_ro/trn_rl_repo"):
    if os.path.isdir(_p) and _p not in sys.path:
        sys.path.append(_p)

import ml_dtypes  # noqa: E402
import concourse.bass as bass  # noqa: E402
import concourse.tile as tile  # noqa: E402
import concourse.mybir as mybir  # noqa: E402
from concourse import bacc  # noqa: E402
from concourse.bass_utils import run_bass_kernel_spmd  # noqa: E402

F32 = mybir.dt.float32
AF = mybir.ActivationFunctionType
STORE = mybir.dt.bfloat16
NP_STORE = ml_dtypes.bfloat16

E = 7
B = 32768
S = 32
A = 8
DIN = S + A + 1        # 40 real rows + ones row (bias via GEMM)
NCORES = 8
BL = B // NCORES       # 4096 batch rows per core
CH = 1024              # batch chunk per psum tile (2 PSUM banks fp32)
NSUB = 512             # one matmul's free dim (1 PSUM bank fp32)
NCHUNK = BL // CH      # 4
NJ = CH // NSUB        # 2
NPS = 4                # psum ring depth: 4 x 2 banks = all 8 banks; hides
                       # drain latency behind 3 tiles of matmul work
NCONST = 8             # const columns per ensemble member

# 1-knot piecewise-quadratic swish fit: s(x) ~ relu(x) - K|x|relu(c-|x|)^2
SW_K = 0.015506
SW_C = 4.9265
SW_S = float(np.sqrt(SW_K))    # layer pre-scale (folded into weights)
SW_C1 = SW_S * SW_C            # knot position in u = SW_S*x units


def _register_swish_op():
    """One 8-op custom DVE instruction: out = relu(u) - |u|*relu(C1-|u|)^2."""
    from concourse import dve_ops as dvo
    from concourse.dve_spec import (
        Spec, Src0, C1, Zero, relu, sq, maxx, lower, _has_src1,
    )
    from concourse.dve_uop import DveOpSpec

    name = "SWISH_SC_ANT"
    for op in dvo.OPS:
        if op.name == name:
            return op
    n = Zero - Src0
    a = maxx(Src0, n)
    body = relu(Src0) - a * sq(relu(C1 - a))
    spec = Spec(
        body=body,
        reference=lambda in0, in1, s0, s1, imm2: (
            np.maximum(in0, 0.0).astype(np.float32)
            - np.abs(in0) * np.maximum(s1 - np.abs(in0), 0.0) ** 2
        ).astype(np.float32),
    )
    row = dvo._CUSTOM_DVE_ROW_BASE + len(dvo.OPS)
    tmp = DveOpSpec(name=name, opcode=row, uops=lower(spec, ver="v3"),
                    rd1_en=_has_src1(spec))
    op = dvo.DveOp(name, spec, subdim=False, uops_sha={"v3": tmp.sha("v3")})
    dvo.OPS.append(op)
    dvo._SUB_OPCODE_FOR_NAME[name] = row
    dvo.CUSTOM_DVE_SPECS[name] = spec
    return op


SWISH_OP = _register_swish_op()


def _build_kernel(ctx, tc, io, act=AF.Silu):
    nc = tc.nc
    cpool = ctx.enter_context(tc.tile_pool(name="cpool", bufs=1))
    hpool = ctx.enter_context(tc.tile_pool(name="hpool", bufs=1))
    # bufs=3: wh of ensemble e-1 stays live through e's ladder (deferred
    # heads) while e+1's weights stream in
    wpool = ctx.enter_context(tc.tile_pool(name="wpool", bufs=3))
    pspool = ctx.enter_context(tc.tile_pool(name="pspool", bufs=NPS, space="PSUM"))
    sgpool = ctx.enter_context(tc.tile_pool(name="sgpool", bufs=3))

    def load_weights(e, first=False):
        # w0 packed for row-tiling: rows 0-40 = W0 cols 0:128 (array rows
        # 0-40), rows 64-104 = W0 cols 128:256 (array rows 64-104); the two
        # matmuls run concurrently in disjoint row groups.
        w0 = wpool.tile([105, 128], STORE, tag="w0", name="w0")
        nc.sync.dma_start(w0[:], io["w0"][e])
        if first:
            # earliest consumers first: xt chunk 0 feeds the prologue L0,
            # cns the first ACT drain; the rest streams behind them
            for j in range(BL // NSUB):
                js = slice(j * NSUB, (j + 1) * NSUB)
                nc.sync.dma_start(xt[0:DIN, js], io["xt"][:, js])
                nc.sync.dma_start(xt[64:64 + DIN, js], io["xt"][:, js])
                if j == 1:
                    nc.sync.dma_start(cns[:], io["cns"])
        b1t = wpool.tile([1, 128], STORE, tag="b1t", name="b1t")
        nc.sync.dma_start(b1t[:], io["b1t"][e])
        if first:
            nc.sync.dma_start(sgc[:], io["sgc"])
        w1, w2, w3 = [], [], []
        for k in range(2):
            t = wpool.tile([128, 256], STORE, tag=f"w1_{k}", name=f"w1_{k}")
            nc.sync.dma_start(t[:], io["w1"][e, k * 128:(k + 1) * 128, :])
            w1.append(t)
            t = wpool.tile([128, 256], STORE, tag=f"w2_{k}", name=f"w2_{k}")
            nc.sync.dma_start(t[:], io["w2"][e, k * 128:(k + 1) * 128, :])
            w2.append(t)
            t = wpool.tile([128, 128], STORE, tag=f"w3_{k}", name=f"w3_{k}")
            nc.sync.dma_start(t[:], io["w3"][e, k * 128:(k + 1) * 128, :])
            w3.append(t)
        wh = wpool.tile([128, 64], STORE, tag="wh", name="wh")
        nc.sync.dma_start(wh[:], io["wh"][e])
        if first:
            # 1 MB residual tensor last: not read until the first head (~25us)
            nc.sync.dma_start(resid[:], io["resid"])
        return w0, w1, w2, w3, wh, b1t

    scratch = cpool.tile([1, 8], F32, tag="scratch")
    nc.gpsimd.memset(scratch[:], 0.0)
    nc.scalar.activation(scratch[0:1, 0:8], scratch[0:1, 0:8], act, bias=0.0)

    # xt duplicated at partitions 64-104 so the row-tiled L0 pair can
    # stream both rhs copies concurrently
    xt = cpool.tile([64 + DIN, BL], STORE, tag="xt")
    cns = cpool.tile([128, E * NCONST], F32, tag="cns")
    sgc = cpool.tile([128, 2], F32, tag="sgc")
    # resid in paired-head layout: rows 0-63 = even 512-col blocks,
    # rows 64-127 = odd blocks (state on rows 0-31 / 64-95)
    resid = cpool.tile([128, BL // 2], F32, tag="resid")
    ones = cpool.tile([1, NSUB], STORE, tag="ones")
    nc.gpsimd.memset(ones[:], 1.0)

    # HAM warm-up: ~4.5us of back-to-back dummy matmuls while the input
    # DMAs stream in, so the PE clock-gate opens (K=8/8) before real work.
    # The psum tile is never read; the ring slot recycles immediately.
    warm = pspool.tile([1, NSUB], F32, tag="ps", name="warm")
    for _ in range(11):
        nc.tensor.matmul(warm[:, :], ones[0:1, 0:1], ones[0:1, :],
                         start=True, stop=True, skip_group_check=True)

    # sigma pre-activations packed 4 ensembles per tile: row 32*(e%4)+i
    pk = [sgpool.tile([128, BL], F32, tag=f"pk{g}", name=f"pk{g}", bufs=1)
          for g in range(2)]

    # --- activation buffers ---
    # h1: double-buffered by ensemble parity (L0(e+1) interleaves with e).
    h1 = [[hpool.tile([128, BL], STORE, tag=f"h1_{p}_{i}", name=f"h1_{p}_{i}")
           for i in range(2)] for p in range(2)]
    h2 = [hpool.tile([128, BL], STORE, tag=f"h2_{i}", name=f"h2_{i}")
          for i in range(2)]
    # h3 double-buffered: heads run one ensemble deferred, so their input
    # drains are long complete and the head matmuls never wait on ACT
    h3 = [hpool.tile([128, BL], STORE, tag=f"h3_{p}", name=f"h3_{p}")
          for p in range(2)]

    def mm_tile(h_in, lhsTs, c, bias_lhsT=None, nsub=NJ):
        """Emit the matmuls for one [128, CH] psum tile; return the tile.

        kt-outer order: consecutive matmuls share their stationary operand,
        so every other LDWEIGHTS hits the already-loaded weight set."""
        ps = pspool.tile([128, CH], F32, tag="ps", name="ps")
        nkt = len(lhsTs)
        if bias_lhsT is not None:
            for j in range(nsub):
                nc.tensor.matmul(ps[:, j * NSUB:(j + 1) * NSUB],
                                 bias_lhsT[:, :], ones[:, :],
                                 start=True, stop=False, skip_group_check=True)
        for kt in range(nkt):
            for j in range(nsub):
                ncol = slice(c * CH + j * NSUB, c * CH + (j + 1) * NSUB)
                nc.tensor.matmul(
                    ps[:, j * NSUB:(j + 1) * NSUB], lhsTs[kt],
                    h_in[kt][:, ncol],
                    start=(bias_lhsT is None and kt == 0),
                    stop=(kt == nkt - 1),
                    skip_group_check=True,
                )
        return ps

    def l0_pair(w0, c, out):
        """Both 128-wide halves of L0 run concurrently as row-tiles (the
        K=41 contraction only occupies rows 0-40 / 64-104 of the array)."""
        psA = pspool.tile([128, CH], F32, tag="ps", name="psA")
        psB = pspool.tile([128, CH], F32, tag="ps", name="psB")
        for j in range(NJ):
            js = slice(j * NSUB, (j + 1) * NSUB)
            ncol = slice(c * CH + j * NSUB, c * CH + (j + 1) * NSUB)
            nc.tensor.matmul(psA[:, js], w0[0:DIN, :], xt[0:DIN, ncol],
                             start=True, stop=True, skip_group_check=True)
            nc.tensor.matmul(psB[:, js], w0[64:64 + DIN, :],
                             xt[64:64 + DIN, ncol],
                             start=True, stop=True, skip_group_check=True)
        drain_dve(psA, out[0][:, c * CH:(c + 1) * CH])
        drain_dve(psB, out[1][:, c * CH:(c + 1) * CH])

    def drain_act(ps, out_ap, bcol):
        nc.scalar.activation(out_ap, ps[:, :], act, bias=cns[:, bcol:bcol + 1])

    def drain_dve(ps, out_ap):
        nc.vector._custom_dve(SWISH_OP, out=out_ap, in0=ps[:, :], s1=SW_C1)

    def head_pair(e, wh, p):
        """Heads for 512-col subchunks 2p and 2p+1 run concurrently as
        col-tiles: psum rows 0-63 = subchunk 2p, rows 64-127 = 2p+1.
        M=64 only fills half the array's columns, so pairing halves the
        head's PE stream time."""
        h3e = h3[e % 2]
        s0 = slice(2 * p * NSUB, (2 * p + 1) * NSUB)
        s1 = slice((2 * p + 1) * NSUB, (2 * p + 2) * NSUB)
        ps = pspool.tile([128, NSUB], F32, tag="ps", name="psh")
        nc.tensor.matmul(ps[0:64, :], wh[:, :], h3e[:, s0],
                         start=True, stop=True, skip_group_check=True)
        nc.tensor.matmul(ps[64:128, :], wh[:, :], h3e[:, s1],
                         start=True, stop=True, skip_group_check=True,
                         tile_position=(0, 64))
        # one fused DVE op drains both head psums (resid is pre-packed in
        # the same row layout):
        #   rows 0-31/64-95:   mu = psum + bmu + state
        #   rows 32-63/96-127: y' = psum + (bsig - max) + 0
        bcol = e * NCONST + 7
        hd = sgpool.tile([128, NSUB], F32, tag="hd", name="hd")
        nc.vector.affine_then_add(
            hd[:, :], ps[:, :], resid[:, p * NSUB:(p + 1) * NSUB], 1.0,
            cns[:, bcol:bcol + 1],
        )
        mu_rows = e * 32
        nc.sync.dma_start(io["mu"][mu_rows:mu_rows + 32, s0], hd[0:32, :])
        nc.sync.dma_start(io["mu"][mu_rows:mu_rows + 32, s1], hd[64:96, :])
        # sigma = 0.5*tanh(y'/2)*exp(max)*2... via the pk pack: members
        # stack 4-wide so each tanh covers up to 128 ACT lanes. SBUF->SBUF
        # DMA does the cross-partition pack (frees the DVE).
        g, r = divmod(e, 4)
        nc.sync.dma_start(pk[g][r * 32:(r + 1) * 32, s0], hd[32:64, :])
        nc.sync.dma_start(pk[g][r * 32:(r + 1) * 32, s1], hd[96:128, :])
        if e in (3, E - 1):
            # flush the pk group: e=3 -> members 0-3, e=6 -> members 4-6
            rows = 128 if e == 3 else 96
            for ss in (s0, s1):
                sg2 = sgpool.tile([128, NSUB], F32, tag="sg2", name="sg2",
                                  bufs=2)
                nc.scalar.activation(sg2[0:rows, :], pk[g][0:rows, ss],
                                     AF.Tanh, scale=0.5)
                sg3 = sgpool.tile([128, NSUB], F32, tag="sg3", name="sg3",
                                  bufs=2)
                nc.vector.tensor_scalar(
                    sg3[0:rows, :], sg2[0:rows, :],
                    sgc[0:rows, 0:1], sgc[0:rows, 1:2],
                    mybir.AluOpType.mult, mybir.AluOpType.add,
                )
                nc.sync.dma_start(io["sig"][g * 128:g * 128 + rows, ss],
                                  sg3[0:rows, :])

    # --- the pipeline ---
    w_cur = None
    for e in range(E):
        if e == 0:
            w_cur = load_weights(0, first=True)
            for c in range(NCHUNK):
                l0_pair(w_cur[0], c, h1[0])
        w0, w1, w2, w3, wh, b1t = w_cur
        h1c = h1[e % 2]
        h1n = h1[(e + 1) % 2]
        cn = e * NCONST
        w_nxt = load_weights(e + 1) if e < E - 1 else None

        def l1_mt0(c):
            ps = mm_tile(h1c, [w1[0][:, 0:128], w1[1][:, 0:128]], c,
                         bias_lhsT=b1t)
            drain_dve(ps, h2[0][:, c * CH:(c + 1) * CH])

        def l1_mt1(c):
            ps = mm_tile(h1c, [w1[0][:, 128:256], w1[1][:, 128:256]], c)
            drain_act(ps, h2[1][:, c * CH:(c + 1) * CH], cn + 3)

        def l0n(c):
            if e == E - 1:
                return
            l0_pair(w_nxt[0], c, h1n)

        def l2(c, mt):
            ps = mm_tile(h2, [w2[0][:, mt * 128:(mt + 1) * 128],
                              w2[1][:, mt * 128:(mt + 1) * 128]], c)
            # L2 writes back into h1c (free after the L1(e) matmuls read it)
            drain_act(ps, h1c[mt][:, c * CH:(c + 1) * CH], cn + 4 + mt)

        def l3(c):
            ps = mm_tile(h1c, [w3[0], w3[1]], c)
            drain_act(ps, h3[e % 2][:, c * CH:(c + 1) * CH], cn + 6)

        # Chunk-pipelined ladder: layer L of chunk c runs alongside layer
        # L+1 of chunk c-1, so DVE drains (l1_mt0/l0n/head) and ACT drains
        # (l1_mt1/l2/l3) alternate evenly and no drain engine ever sees a
        # long same-engine burst. Heads of e-1 ride this ensemble's ladder:
        # their h3 drained an ensemble ago, so the head matmuls never wait.
        for c in range(NCHUNK):
            l1_mt0(c)                               # D
            l1_mt1(c)                               # A
            if e > 0:
                head_pair(e - 1, wh_prev, c)        # D
            l0n(c)                                  # DD
            if c >= 1:
                l2(c - 1, 0)                        # A
                l2(c - 1, 1)                        # A
            if c >= 2:
                l3(c - 2)                           # A
        l2(NCHUNK - 1, 0)                           # A
        l2(NCHUNK - 1, 1)                           # A
        l3(NCHUNK - 2)                              # A
        l3(NCHUNK - 1)                              # A
        wh_prev = wh
        w_cur = w_nxt

    # epilogue: the last ensemble's heads (+ the pk group-1 sigma flush)
    for p in range(BL // (2 * NSUB)):
        head_pair(E - 1, wh_prev, p)


def build_program(act=AF.Silu):
    nc = bacc.Bacc(
        "TRN2", target_bir_lowering=False, debug=False, num_devices=NCORES
    )
    io = {
        "xt": nc.dram_tensor("xt", [DIN, BL], STORE,
                             kind="ExternalInput").ap(),
        "resid": nc.dram_tensor("resid", [128, BL // 2], F32,
                                kind="ExternalInput").ap(),
        "w0": nc.dram_tensor("w0", [E, 105, 128], STORE,
                             kind="ExternalInput").ap(),
        "w1": nc.dram_tensor("w1", [E, 256, 256], STORE,
                             kind="ExternalInput").ap(),
        "w2": nc.dram_tensor("w2", [E, 256, 256], STORE,
                             kind="ExternalInput").ap(),
        "w3": nc.dram_tensor("w3", [E, 256, 128], STORE,
                             kind="ExternalInput").ap(),
        "wh": nc.dram_tensor("wh", [E, 128, 64], STORE,
                             kind="ExternalInput").ap(),
        "b1t": nc.dram_tensor("b1t", [E, 1, 128], STORE,
                              kind="ExternalInput").ap(),
        "cns": nc.dram_tensor("cns", [128, E * NCONST], F32,
                              kind="ExternalInput").ap(),
        "sgc": nc.dram_tensor("sgc", [128, 2], F32, kind="ExternalInput").ap(),
        "mu": nc.dram_tensor("mu", [E * 32, BL], F32,
                             kind="ExternalOutput").ap(),
        "sig": nc.dram_tensor("sig", [E * 32, BL], F32,
                              kind="ExternalOutput").ap(),
    }
    with tile.TileContext(nc) as tc, ExitStack() as ctx:
        _build_kernel(ctx, tc, io, act=act)
    nc.compile()
    return nc


def host_prep(state, action, W0, b0, W1, b1, W2, b2, W3, b3,
              Wmu, bmu, Wsig, bsig, max_logstd, min_logstd):
    """Full inputs -> (shared input map, per-core shard maps)."""
    f = lambda a: np.ascontiguousarray(np.asarray(a), dtype=np.float32)
    g = lambda a: np.ascontiguousarray(np.asarray(a, dtype=np.float32)
                                       .astype(NP_STORE))
    state, action = f(state), f(action)
    x_full = np.concatenate([state, action], axis=1)          # [B, 40]
    xt_full = np.concatenate(
        [x_full.T, np.ones((1, B), np.float32)], axis=0
    )  # [41, B] with the ones row for GEMM-side biases
    W0, W1, W2, W3 = f(W0), f(W1), f(W2), f(W3)
    b0, b1, b2, b3 = f(b0), f(b1), f(b2), f(b3)
    bmu, bsig = f(bmu), f(bsig)
    mx, mn = f(max_logstd), f(min_logstd)

    s = SW_S
    # L0 folds its bias into the ones-row and pre-scales by s; the DVE swish
    # then emits s*h1, compensated in W1. L1's mt0 half does the same (its
    # bias rides a K=1 matmul), compensated in W2's first k-tile.
    W0a = np.concatenate([W0, b0[:, None, :]], axis=1) * s     # [E,41,256]
    # packed for row-tiled pairing: rows 0-40 = cols 0:128, rows 64-104 =
    # cols 128:256
    W0p = np.zeros((E, 105, 128), np.float32)
    W0p[:, 0:41, :] = W0a[:, :, 0:128]
    W0p[:, 64:105, :] = W0a[:, :, 128:256]
    W1p = np.concatenate([W1[:, :, 0:128],                     # *(s/s) = 1
                          W1[:, :, 128:256] / s], axis=2)
    b1t = (b1[:, None, 0:128] * s)                             # [E,1,128]
    W2p = np.concatenate([W2[:, 0:128, :] / s, W2[:, 128:256, :]], axis=1)
    wh = np.concatenate([f(Wmu), f(Wsig)], axis=2)

    cns = np.zeros((128, E * NCONST), np.float32)
    for e in range(E):
        c = e * NCONST
        cns[:, c + 3] = b1[e, 128:]
        cns[:, c + 4] = b2[e, :128]
        cns[:, c + 5] = b2[e, 128:]
        cns[:, c + 6] = b3[e, :]
        cns[0:32, c + 7] = bmu[e]
        cns[32:64, c + 7] = bsig[e] - mx   # sigma-head drain bias
        cns[64:96, c + 7] = bmu[e]         # paired-head upper half
        cns[96:128, c + 7] = bsig[e] - mx

    sgc = np.zeros((128, 2), np.float32)
    sgc[:, 0] = np.tile(np.exp(mx) / 2, 4)
    sgc[:, 1] = np.tile(np.exp(mn) + np.exp(mx) / 2, 4)

    shared = {
        "w0": g(W0p), "w1": g(W1p), "w2": g(W2p), "w3": g(W3),
        "wh": g(wh), "b1t": g(b1t), "cns": cns, "sgc": sgc,
    }
    # resid in paired-head layout: [128, B/2] with even 512-col blocks on
    # rows 0-31 and odd blocks on rows 64-95 (state only; sigma rows stay 0)
    st = xt_full[0:32].reshape(32, B // 512, 512)
    resid_full = np.zeros((128, B // 2), np.float32)
    resid_full[0:32] = st[:, 0::2, :].reshape(32, B // 2)
    resid_full[64:96] = st[:, 1::2, :].reshape(32, B // 2)
    xt_store = xt_full.astype(NP_STORE)
    hb = BL // 2
    shards = [
        {
            "xt": np.ascontiguousarray(xt_store[:, c * BL:(c + 1) * BL]),
            "resid": np.ascontiguousarray(resid_full[:, c * hb:(c + 1) * hb]),
        }
        for c in range(NCORES)
    ]
    return shared, shards


def host_post(results):
    """Per-core {mu,sig} [E*32, BL] -> (mu [E,B,32], sigma [E,B,32])."""
    mu = np.empty((E, B, 32), np.float32)
    sigma = np.empty((E, B, 32), np.float32)
    for c in range(NCORES):
        bs = slice(c * BL, (c + 1) * BL)
        mu[:, bs, :] = results[c]["mu"].reshape(E, 32, BL).transpose(0, 2, 1)
        sigma[:, bs, :] = results[c]["sig"].reshape(E, 32, BL).transpose(0, 2, 1)
    return mu, sigma


_PROGRAM = None


def _get_program():
    global _PROGRAM
    if _PROGRAM is None:
        _PROGRAM = build_program()
    return _PROGRAM


def kernel(**inputs):
    nc = _get_program()
    shared, shards = host_prep(**inputs)
    in_maps = [{**shared, **shards[c]} for c in range(NCORES)]
    res = run_bass_kernel_spmd(nc, in_maps, list(range(NCORES)))
    return host_post(res.results)


# revision 33
# speedup vs baseline: 1.2096x; 1.0024x over previous
"""Trainium2 Bass kernel for nn_EnsembleDynamicModel.

Ensemble MLP: E=7 members, x=[state(32)|action(8)] -> 256 -> 256 -> 256 -> 128
-> {mu(32), log_sigma(32)} with swish hidden activations, soft-clamped
log_sigma -> sigma=exp(.), and mu += state residual.

Strategy: data-parallel over the batch axis; 8 cores x 4096 rows, weights
replicated. Activations are feature-major ([feature, batch]) so every GEMM's
contraction dim sits on SBUF partitions.

The scalar (ACT) engine is the intrinsic bottleneck of a pure-ACT swish
design: 25.7M hidden elements/core at 1 elem/cycle/lane @1.2GHz = 167us,
above the PE's ~157us of matmul work, so engines must share the swish load.
A custom 8-op DVE instruction computes an approximate swish in one pass:

    out = relu(u) - |u| * relu(C1 - |u|)^2      (u = sqrt(K) * x)

which equals sqrt(K)*(relu(x) - K*|x|*relu(c-|x|)^2), a 1-knot piecewise-
quadratic swish (max abs err 0.039, and preacts here stay in |x|<3.6 where
the fit is tight; end-to-end scale_rel ~2.5e-3 vs the 2e-2 gate). The
sqrt(K) scales fold into the host-side weights: this layer's W,b scaled up,
next layer's W scaled down, so the hardware op needs zero spare ALU stages
for scaling.

Work split per ensemble (per core):
  DVE : L0's 4 drain tiles (bias rides a ones-row: K=40->41 costs nothing,
        matmul time is N-driven) + L1's mt0 tiles (bias injected via a K=1
        ones-matmul into PSUM) + the head affine drains.  ~19us
  ACT : L1 mt1 + L2 + L3 exact Silu drains + the sigma tanh.  ~20us
  PE  : the GEMM chain + K=1 bias matmuls.  ~24us  <- critical engine
  DMA : weights/input/output + the sigma pk pack copies (SBUF->SBUF DMA
        replaces cross-partition DVE copies).

Drains alternate DVE/ACT down the ensemble so no engine sees a >2-tile
burst and the PE (2-deep PSUM ring) never waits on a busy drain engine;
L0(e+1) is interleaved into L1/L2(e) to fill the PE's drain-latency gaps.
"""

import os
import sys
import numpy as np
from contextlib import ExitStack

for _p in ("/opt/trn_rl_repo", "/root/.axon_site/_ro/trn_rl_repo"):
    if os.path.isdir(_p) and _p not in sys.path:
        sys.path.append(_p)

import ml_dtypes  # noqa: E402
import concourse.bass as bass  # noqa: E402
import concourse.tile as tile  # noqa: E402
import concourse.mybir as mybir  # noqa: E402
from concourse import bacc  # noqa: E402
from concourse.bass_utils import run_bass_kernel_spmd  # noqa: E402

F32 = mybir.dt.float32
AF = mybir.ActivationFunctionType
STORE = mybir.dt.bfloat16
NP_STORE = ml_dtypes.bfloat16

E = 7
B = 32768
S = 32
A = 8
DIN = S + A + 1        # 40 real rows + ones row (bias via GEMM)
NCORES = 8
BL = B // NCORES       # 4096 batch rows per core
CH = 1024              # batch chunk per psum tile (2 PSUM banks fp32)
NSUB = 512             # one matmul's free dim (1 PSUM bank fp32)
NCHUNK = BL // CH      # 4
NJ = CH // NSUB        # 2
NPS = 4                # psum ring depth: 4 x 2 banks = all 8 banks; hides
                       # drain latency behind 3 tiles of matmul work
NCONST = 8             # const columns per ensemble member

# 1-knot piecewise-quadratic swish fit: s(x) ~ relu(x) - K|x|relu(c-|x|)^2
SW_K = 0.015506
SW_C = 4.9265
SW_S = float(np.sqrt(SW_K))    # layer pre-scale (folded into weights)
SW_C1 = SW_S * SW_C            # knot position in u = SW_S*x units


def _register_swish_op():
    """One 8-op custom DVE instruction: out = relu(u) - |u|*relu(C1-|u|)^2."""
    from concourse import dve_ops as dvo
    from concourse.dve_spec import (
        Spec, Src0, C1, Zero, relu, sq, maxx, lower, _has_src1,
    )
    from concourse.dve_uop import DveOpSpec

    name = "SWISH_SC_ANT"
    for op in dvo.OPS:
        if op.name == name:
            return op
    n = Zero - Src0
    a = maxx(Src0, n)
    body = relu(Src0) - a * sq(relu(C1 - a))
    spec = Spec(
        body=body,
        reference=lambda in0, in1, s0, s1, imm2: (
            np.maximum(in0, 0.0).astype(np.float32)
            - np.abs(in0) * np.maximum(s1 - np.abs(in0), 0.0) ** 2
        ).astype(np.float32),
    )
    row = dvo._CUSTOM_DVE_ROW_BASE + len(dvo.OPS)
    tmp = DveOpSpec(name=name, opcode=row, uops=lower(spec, ver="v3"),
                    rd1_en=_has_src1(spec))
    op = dvo.DveOp(name, spec, subdim=False, uops_sha={"v3": tmp.sha("v3")})
    dvo.OPS.append(op)
    dvo._SUB_OPCODE_FOR_NAME[name] = row
    dvo.CUSTOM_DVE_SPECS[name] = spec
    return op


SWISH_OP = _register_swish_op()


def _build_kernel(ctx, tc, io, act=AF.Silu):
    nc = tc.nc
    cpool = ctx.enter_context(tc.tile_pool(name="cpool", bufs=1))
    hpool = ctx.enter_context(tc.tile_pool(name="hpool", bufs=1))
    # bufs=3: wh of ensemble e-1 stays live through e's ladder (deferred
    # heads) while e+1's weights stream in
    wpool = ctx.enter_context(tc.tile_pool(name="wpool", bufs=3))
    pspool = ctx.enter_context(tc.tile_pool(name="pspool", bufs=NPS, space="PSUM"))
    sgpool = ctx.enter_context(tc.tile_pool(name="sgpool", bufs=3))

    def load_weights(e, first=False):
        # w0 packed for row-tiling: rows 0-40 = W0 cols 0:128 (array rows
        # 0-40), rows 64-104 = W0 cols 128:256 (array rows 64-104); the two
        # matmuls run concurrently in disjoint row groups.
        w0 = wpool.tile([105, 128], STORE, tag="w0", name="w0")
        nc.sync.dma_start(w0[:], io["w0"][e])
        if first:
            # two full-width transfers: one big DMA sustains full bandwidth
            # (~1us each), where 16 chunked ones trickle in at queue rate
            # and starve the L0 prologue
            nc.sync.dma_start(xt[0:DIN, :], io["xt"][:, :])
            nc.sync.dma_start(xt[64:64 + DIN, :], io["xt"][:, :])
            nc.sync.dma_start(cns[:], io["cns"])
        b1t = wpool.tile([1, 128], STORE, tag="b1t", name="b1t")
        nc.sync.dma_start(b1t[:], io["b1t"][e])
        if first:
            nc.sync.dma_start(sgc[:], io["sgc"])
        w1, w2, w3 = [], [], []
        for k in range(2):
            t = wpool.tile([128, 256], STORE, tag=f"w1_{k}", name=f"w1_{k}")
            nc.sync.dma_start(t[:], io["w1"][e, k * 128:(k + 1) * 128, :])
            w1.append(t)
            t = wpool.tile([128, 256], STORE, tag=f"w2_{k}", name=f"w2_{k}")
            nc.sync.dma_start(t[:], io["w2"][e, k * 128:(k + 1) * 128, :])
            w2.append(t)
            t = wpool.tile([128, 128], STORE, tag=f"w3_{k}", name=f"w3_{k}")
            nc.sync.dma_start(t[:], io["w3"][e, k * 128:(k + 1) * 128, :])
            w3.append(t)
        wh = wpool.tile([128, 64], STORE, tag="wh", name="wh")
        nc.sync.dma_start(wh[:], io["wh"][e])
        if first:
            # 1 MB residual tensor last: not read until the first head (~25us)
            nc.sync.dma_start(resid[:], io["resid"])
        return w0, w1, w2, w3, wh, b1t

    scratch = cpool.tile([1, 8], F32, tag="scratch")
    nc.gpsimd.memset(scratch[:], 0.0)
    nc.scalar.activation(scratch[0:1, 0:8], scratch[0:1, 0:8], act, bias=0.0)

    # xt duplicated at partitions 64-104 so the row-tiled L0 pair can
    # stream both rhs copies concurrently
    xt = cpool.tile([64 + DIN, BL], STORE, tag="xt")
    cns = cpool.tile([128, E * NCONST], F32, tag="cns")
    sgc = cpool.tile([128, 2], F32, tag="sgc")
    # resid in paired-head layout: rows 0-63 = even 512-col blocks,
    # rows 64-127 = odd blocks (state on rows 0-31 / 64-95)
    resid = cpool.tile([128, BL // 2], F32, tag="resid")
    ones = cpool.tile([1, NSUB], STORE, tag="ones")
    nc.gpsimd.memset(ones[:], 1.0)

    # HAM warm-up: ~4.5us of back-to-back dummy matmuls while the input
    # DMAs stream in, so the PE clock-gate opens (K=8/8) before real work.
    # The psum tile is never read; the ring slot recycles immediately.
    warm = pspool.tile([1, NSUB], F32, tag="ps", name="warm")
    for _ in range(11):
        nc.tensor.matmul(warm[:, :], ones[0:1, 0:1], ones[0:1, :],
                         start=True, stop=True, skip_group_check=True)

    # sigma pre-activations packed 4 ensembles per tile: row 32*(e%4)+i
    pk = [sgpool.tile([128, BL], F32, tag=f"pk{g}", name=f"pk{g}", bufs=1)
          for g in range(2)]

    # --- activation buffers ---
    # h1: double-buffered by ensemble parity (L0(e+1) interleaves with e).
    h1 = [[hpool.tile([128, BL], STORE, tag=f"h1_{p}_{i}", name=f"h1_{p}_{i}")
           for i in range(2)] for p in range(2)]
    h2 = [hpool.tile([128, BL], STORE, tag=f"h2_{i}", name=f"h2_{i}")
          for i in range(2)]
    # h3 double-buffered: heads run one ensemble deferred, so their input
    # drains are long complete and the head matmuls never wait on ACT
    h3 = [hpool.tile([128, BL], STORE, tag=f"h3_{p}", name=f"h3_{p}")
          for p in range(2)]

    def mm_tile(h_in, lhsTs, c, bias_lhsT=None, nsub=NJ):
        """Emit the matmuls for one [128, CH] psum tile; return the tile.

        kt-outer order: consecutive matmuls share their stationary operand,
        so every other LDWEIGHTS hits the already-loaded weight set."""
        ps = pspool.tile([128, CH], F32, tag="ps", name="ps")
        nkt = len(lhsTs)
        if bias_lhsT is not None:
            for j in range(nsub):
                nc.tensor.matmul(ps[:, j * NSUB:(j + 1) * NSUB],
                                 bias_lhsT[:, :], ones[:, :],
                                 start=True, stop=False, skip_group_check=True)
        for kt in range(nkt):
            for j in range(nsub):
                ncol = slice(c * CH + j * NSUB, c * CH + (j + 1) * NSUB)
                nc.tensor.matmul(
                    ps[:, j * NSUB:(j + 1) * NSUB], lhsTs[kt],
                    h_in[kt][:, ncol],
                    start=(bias_lhsT is None and kt == 0),
                    stop=(kt == nkt - 1),
                    skip_group_check=True,
                )
        return ps

    def l0_pair(w0, c, out):
        """Both 128-wide halves of L0 run concurrently as row-tiles (the
        K=41 contraction only occupies rows 0-40 / 64-104 of the array)."""
        psA = pspool.tile([128, CH], F32, tag="ps", name="psA")
        psB = pspool.tile([128, CH], F32, tag="ps", name="psB")
        for j in range(NJ):
            js = slice(j * NSUB, (j + 1) * NSUB)
            ncol = slice(c * CH + j * NSUB, c * CH + (j + 1) * NSUB)
            nc.tensor.matmul(psA[:, js], w0[0:DIN, :], xt[0:DIN, ncol],
                             start=True, stop=True, skip_group_check=True)
            nc.tensor.matmul(psB[:, js], w0[64:64 + DIN, :],
                             xt[64:64 + DIN, ncol],
                             start=True, stop=True, skip_group_check=True)
        drain_dve(psA, out[0][:, c * CH:(c + 1) * CH])
        drain_dve(psB, out[1][:, c * CH:(c + 1) * CH])

    def drain_act(ps, out_ap, bcol):
        nc.scalar.activation(out_ap, ps[:, :], act, bias=cns[:, bcol:bcol + 1])

    def drain_dve(ps, out_ap):
        nc.vector._custom_dve(SWISH_OP, out=out_ap, in0=ps[:, :], s1=SW_C1)

    def head_pair(e, wh, p):
        """Heads for 512-col subchunks 2p and 2p+1 run concurrently as
        col-tiles: psum rows 0-63 = subchunk 2p, rows 64-127 = 2p+1.
        M=64 only fills half the array's columns, so pairing halves the
        head's PE stream time."""
        h3e = h3[e % 2]
        s0 = slice(2 * p * NSUB, (2 * p + 1) * NSUB)
        s1 = slice((2 * p + 1) * NSUB, (2 * p + 2) * NSUB)
        ps = pspool.tile([128, NSUB], F32, tag="ps", name="psh")
        nc.tensor.matmul(ps[0:64, :], wh[:, :], h3e[:, s0],
                         start=True, stop=True, skip_group_check=True)
        nc.tensor.matmul(ps[64:128, :], wh[:, :], h3e[:, s1],
                         start=True, stop=True, skip_group_check=True,
                         tile_position=(0, 64))
        # one fused DVE op drains both head psums (resid is pre-packed in
        # the same row layout):
        #   rows 0-31/64-95:   mu = psum + bmu + state
        #   rows 32-63/96-127: y' = psum + (bsig - max) + 0
        bcol = e * NCONST + 7
        hd = sgpool.tile([128, NSUB], F32, tag="hd", name="hd")
        nc.vector.affine_then_add(
            hd[:, :], ps[:, :], resid[:, p * NSUB:(p + 1) * NSUB], 1.0,
            cns[:, bcol:bcol + 1],
        )
        mu_rows = e * 32
        nc.sync.dma_start(io["mu"][mu_rows:mu_rows + 32, s0], hd[0:32, :])
        nc.sync.dma_start(io["mu"][mu_rows:mu_rows + 32, s1], hd[64:96, :])
        # sigma = 0.5*tanh(y'/2)*exp(max)*2... via the pk pack: members
        # stack 4-wide so each tanh covers up to 128 ACT lanes. SBUF->SBUF
        # DMA does the cross-partition pack (frees the DVE).
        g, r = divmod(e, 4)
        nc.sync.dma_start(pk[g][r * 32:(r + 1) * 32, s0], hd[32:64, :])
        nc.sync.dma_start(pk[g][r * 32:(r + 1) * 32, s1], hd[96:128, :])
        if e in (3, E - 1):
            # flush the pk group: e=3 -> members 0-3, e=6 -> members 4-6
            rows = 128 if e == 3 else 96
            for ss in (s0, s1):
                sg2 = sgpool.tile([128, NSUB], F32, tag="sg2", name="sg2",
                                  bufs=2)
                nc.scalar.activation(sg2[0:rows, :], pk[g][0:rows, ss],
                                     AF.Tanh, scale=0.5)
                sg3 = sgpool.tile([128, NSUB], F32, tag="sg3", name="sg3",
                                  bufs=2)
                nc.vector.tensor_scalar(
                    sg3[0:rows, :], sg2[0:rows, :],
                    sgc[0:rows, 0:1], sgc[0:rows, 1:2],
                    mybir.AluOpType.mult, mybir.AluOpType.add,
                )
                nc.sync.dma_start(io["sig"][g * 128:g * 128 + rows, ss],
                                  sg3[0:rows, :])

    # --- the pipeline ---
    w_cur = None
    for e in range(E):
        if e == 0:
            w_cur = load_weights(0, first=True)
            for c in range(NCHUNK):
                l0_pair(w_cur[0], c, h1[0])
        w0, w1, w2, w3, wh, b1t = w_cur
        h1c = h1[e % 2]
        h1n = h1[(e + 1) % 2]
        cn = e * NCONST
        w_nxt = load_weights(e + 1) if e < E - 1 else None

        def l1_mt0(c):
            ps = mm_tile(h1c, [w1[0][:, 0:128], w1[1][:, 0:128]], c,
                         bias_lhsT=b1t)
            drain_dve(ps, h2[0][:, c * CH:(c + 1) * CH])

        def l1_mt1(c):
            ps = mm_tile(h1c, [w1[0][:, 128:256], w1[1][:, 128:256]], c)
            drain_act(ps, h2[1][:, c * CH:(c + 1) * CH], cn + 3)

        def l0n(c):
            if e == E - 1:
                return
            l0_pair(w_nxt[0], c, h1n)

        def l2(c, mt):
            ps = mm_tile(h2, [w2[0][:, mt * 128:(mt + 1) * 128],
                              w2[1][:, mt * 128:(mt + 1) * 128]], c)
            # L2 writes back into h1c (free after the L1(e) matmuls read it)
            drain_act(ps, h1c[mt][:, c * CH:(c + 1) * CH], cn + 4 + mt)

        def l3(c):
            ps = mm_tile(h1c, [w3[0], w3[1]], c)
            drain_act(ps, h3[e % 2][:, c * CH:(c + 1) * CH], cn + 6)

        # Chunk-pipelined ladder: layer L of chunk c runs alongside layer
        # L+1 of chunk c-1, so DVE drains (l1_mt0/l0n/head) and ACT drains
        # (l1_mt1/l2/l3) alternate evenly and no drain engine ever sees a
        # long same-engine burst. Heads of e-1 ride this ensemble's ladder:
        # their h3 drained an ensemble ago, so the head matmuls never wait.
        for c in range(NCHUNK):
            l1_mt0(c)                               # D
            l1_mt1(c)                               # A
            if e > 0:
                head_pair(e - 1, wh_prev, c)        # D
            l0n(c)                                  # DD
            if c >= 1:
                l2(c - 1, 0)                        # A
                l2(c - 1, 1)                        # A
            if c >= 2:
                l3(c - 2)                           # A
        l2(NCHUNK - 1, 0)                           # A
        l2(NCHUNK - 1, 1)                           # A
        l3(NCHUNK - 2)                              # A
        l3(NCHUNK - 1)                              # A
        wh_prev = wh
        w_cur = w_nxt

    # epilogue: the last ensemble's heads (+ the pk group-1 sigma flush)
    for p in range(BL // (2 * NSUB)):
        head_pair(E - 1, wh_prev, p)


def build_program(act=AF.Silu):
    nc = bacc.Bacc(
        "TRN2", target_bir_lowering=False, debug=False, num_devices=NCORES
    )
    io = {
        "xt": nc.dram_tensor("xt", [DIN, BL], STORE,
                             kind="ExternalInput").ap(),
        "resid": nc.dram_tensor("resid", [128, BL // 2], F32,
                                kind="ExternalInput").ap(),
        "w0": nc.dram_tensor("w0", [E, 105, 128], STORE,
                             kind="ExternalInput").ap(),
        "w1": nc.dram_tensor("w1", [E, 256, 256], STORE,
                             kind="ExternalInput").ap(),
        "w2": nc.dram_tensor("w2", [E, 256, 256], STORE,
                             kind="ExternalInput").ap(),
        "w3": nc.dram_tensor("w3", [E, 256, 128], STORE,
                             kind="ExternalInput").ap(),
        "wh": nc.dram_tensor("wh", [E, 128, 64], STORE,
                             kind="ExternalInput").ap(),
        "b1t": nc.dram_tensor("b1t", [E, 1, 128], STORE,
                              kind="ExternalInput").ap(),
        "cns": nc.dram_tensor("cns", [128, E * NCONST], F32,
                              kind="ExternalInput").ap(),
        "sgc": nc.dram_tensor("sgc", [128, 2], F32, kind="ExternalInput").ap(),
        "mu": nc.dram_tensor("mu", [E * 32, BL], F32,
                             kind="ExternalOutput").ap(),
        "sig": nc.dram_tensor("sig", [E * 32, BL], F32,
                              kind="ExternalOutput").ap(),
    }
    with tile.TileContext(nc) as tc, ExitStack() as ctx:
        _build_kernel(ctx, tc, io, act=act)
    nc.compile()
    return nc


def host_prep(state, action, W0, b0, W1, b1, W2, b2, W3, b3,
              Wmu, bmu, Wsig, bsig, max_logstd, min_logstd):
    """Full inputs -> (shared input map, per-core shard maps)."""
    f = lambda a: np.ascontiguousarray(np.asarray(a), dtype=np.float32)
    g = lambda a: np.ascontiguousarray(np.asarray(a, dtype=np.float32)
                                       .astype(NP_STORE))
    state, action = f(state), f(action)
    x_full = np.concatenate([state, action], axis=1)          # [B, 40]
    xt_full = np.concatenate(
        [x_full.T, np.ones((1, B), np.float32)], axis=0
    )  # [41, B] with the ones row for GEMM-side biases
    W0, W1, W2, W3 = f(W0), f(W1), f(W2), f(W3)
    b0, b1, b2, b3 = f(b0), f(b1), f(b2), f(b3)
    bmu, bsig = f(bmu), f(bsig)
    mx, mn = f(max_logstd), f(min_logstd)

    s = SW_S
    # L0 folds its bias into the ones-row and pre-scales by s; the DVE swish
    # then emits s*h1, compensated in W1. L1's mt0 half does the same (its
    # bias rides a K=1 matmul), compensated in W2's first k-tile.
    W0a = np.concatenate([W0, b0[:, None, :]], axis=1) * s     # [E,41,256]
    # packed for row-tiled pairing: rows 0-40 = cols 0:128, rows 64-104 =
    # cols 128:256
    W0p = np.zeros((E, 105, 128), np.float32)
    W0p[:, 0:41, :] = W0a[:, :, 0:128]
    W0p[:, 64:105, :] = W0a[:, :, 128:256]
    W1p = np.concatenate([W1[:, :, 0:128],                     # *(s/s) = 1
                          W1[:, :, 128:256] / s], axis=2)
    b1t = (b1[:, None, 0:128] * s)                             # [E,1,128]
    W2p = np.concatenate([W2[:, 0:128, :] / s, W2[:, 128:256, :]], axis=1)
    wh = np.concatenate([f(Wmu), f(Wsig)], axis=2)

    cns = np.zeros((128, E * NCONST), np.float32)
    for e in range(E):
        c = e * NCONST
        cns[:, c + 3] = b1[e, 128:]
        cns[:, c + 4] = b2[e, :128]
        cns[:, c + 5] = b2[e, 128:]
        cns[:, c + 6] = b3[e, :]
        cns[0:32, c + 7] = bmu[e]
        cns[32:64, c + 7] = bsig[e] - mx   # sigma-head drain bias
        cns[64:96, c + 7] = bmu[e]         # paired-head upper half
        cns[96:128, c + 7] = bsig[e] - mx

    sgc = np.zeros((128, 2), np.float32)
    sgc[:, 0] = np.tile(np.exp(mx) / 2, 4)
    sgc[:, 1] = np.tile(np.exp(mn) + np.exp(mx) / 2, 4)

    shared = {
        "w0": g(W0p), "w1": g(W1p), "w2": g(W2p), "w3": g(W3),
        "wh": g(wh), "b1t": g(b1t), "cns": cns, "sgc": sgc,
    }
    # resid in paired-head layout: [128, B/2] with even 512-col blocks on
    # rows 0-31 and odd blocks on rows 64-95 (state only; sigma rows stay 0)
    st = xt_full[0:32].reshape(32, B // 512, 512)
    resid_full = np.zeros((128, B // 2), np.float32)
    resid_full[0:32] = st[:, 0::2, :].reshape(32, B // 2)
    resid_full[64:96] = st[:, 1::2, :].reshape(32, B // 2)
    xt_store = xt_full.astype(NP_STORE)
    hb = BL // 2
    shards = [
        {
            "xt": np.ascontiguousarray(xt_store[:, c * BL:(c + 1) * BL]),
            "resid": np.ascontiguousarray(resid_full[:, c * hb:(c + 1) * hb]),
        }
        for c in range(NCORES)
    ]
    return shared, shards


def host_post(results):
    """Per-core {mu,sig} [E*32, BL] -> (mu [E,B,32], sigma [E,B,32])."""
    mu = np.empty((E, B, 32), np.float32)
    sigma = np.empty((E, B, 32), np.float32)
    for c in range(NCORES):
        bs = slice(c * BL, (c + 1) * BL)
        mu[:, bs, :] = results[c]["mu"].reshape(E, 32, BL).transpose(0, 2, 1)
        sigma[:, bs, :] = results[c]["sig"].reshape(E, 32, BL).transpose(0, 2, 1)
    return mu, sigma


_PROGRAM = None


def _get_program():
    global _PROGRAM
    if _PROGRAM is None:
        _PROGRAM = build_program()
    return _PROGRAM


def kernel(**inputs):
    nc = _get_program()
    shared, shards = host_prep(**inputs)
    in_maps = [{**shared, **shards[c]} for c in range(NCORES)]
    res = run_bass_kernel_spmd(nc, in_maps, list(range(NCORES)))
    return host_post(res.results)
